# revision 55
# baseline (speedup 1.0000x reference)
# GAT decoder kernel for Trainium2 (8 NeuronCores, SPMD, no collectives).
#
# Every core redundantly computes the fp16 MLP (z -> x1 -> x2 -> xpx) for ALL
# 20000 nodes and writes a private node-major gather table `tab` in its own
# HBM. Each core receives z pre-rotated (its own 2500 nodes first) and
# pre-transposed on the host, so all tab/loc offsets are core-independent and
# the instruction stream is SPMD-uniform. This removes the AllGather entirely.
#
# Table row (fp16, 384 elems / 768B stride):
#   [0:128]  xp head0            [128]   1.0 (denominator column h0)
#   [129:257] xp head1           [257]   1.0 (denominator column h1)
#   [258:260] a_src (h0,h1)      [260:262] a_dst (h0,h1)   [262:384] pad
#
# Edge phase per 128-dst super-chunk (sc): dma_gather pulls 768B rows per
# edge-slot into G; DVE/ACT build S = mask * exp(leaky_relu(a_src+a_dst));
# PE accumulates S^T @ [xp_h|1] (messages + denominator in one 129-wide
# matmul) into PSUM per 32-dst window. Chunks are host-planned with uniform
# per-sc caps (identical stream across cores) and edges sorted by source row
# inside each window for HBM locality.
import numpy as np

LATENT, HID, OUT, HEADS = 512, 256, 128, 2
NEG = 0.2
N_CORES = 8
W = 32           # dst rows per window
NWIN = 4         # windows per super-chunk
GMAX = 8         # chunks per dma_gather call (<=1024 idx HW limit)
ROW = 384        # fp16 elements per gather-table row (768 bytes)
C = HEADS * OUT  # 256 message channels
LAST_RESULT = None


def _plan(src, dst, N, NB, n_sc):
    """Chunk plan: per sc, NWIN windows with a uniform cap (max over cores and
    windows of ceil(cnt/128)). Within each window edges are sorted by rotated
    source row and dealt into consecutive chunks."""
    counts = np.zeros((N_CORES, n_sc, NWIN), np.int64)
    per_core = []
    for c in range(N_CORES):
        sel = (dst >= c * NB) & (dst < (c + 1) * NB)
        d = dst[sel] - c * NB
        s_rot = (src[sel] - c * NB) % N
        sc = d // 128
        wi = (d % 128) // W
        counts[c] = np.stack([np.bincount((sc[wi == w]) , minlength=n_sc)
                              for w in range(NWIN)], axis=1)
        per_core.append((d, s_rot, sc, wi))

    caps = np.maximum(1, np.ceil(counts.max(axis=0) / 128.0).astype(np.int64).max(axis=1))
    sc_base = np.concatenate([[0], np.cumsum(NWIN * caps)])
    NCH = int(sc_base[-1])

    gidx_all, mask_all = [], []
    for c in range(N_CORES):
        d, s_rot, sc, wi = per_core[c]
        gidx = np.zeros(NCH * 128, np.int16)
        mask = np.zeros((128, NCH, W), np.float16)
        o = np.lexsort((s_rot, wi, sc))
        d, s_rot, sc, wi = d[o], s_rot[o], sc[o], wi[o]
        # rank within each (sc, wi) group
        grp = sc * NWIN + wi
        start = np.zeros(n_sc * NWIN + 1, np.int64)
        np.add.at(start, grp + 1, 1)
        start = np.cumsum(start)
        r = np.arange(len(d)) - start[grp]
        k = sc_base[sc] + wi * caps[sc] + r // 128
        p = r % 128
        gidx[k * 128 + p] = s_rot.astype(np.int16)
        mask[p, k, d % W] = 1.0
        i = np.arange(NCH * 128)
        gw = np.zeros((128, NCH * 8), np.int16)
        gw[(i % 16)[None, :] + 16 * np.arange(8)[:, None], (i // 16)[None, :]] = gidx[None, :]
        gidx_all.append(gw)
        mask_all.append(mask)
    return gidx_all, mask_all, caps, sc_base, NCH


def _build(N, NB, n_sc, caps, sc_base, NCH):
    import concourse.bass as bass
    import concourse.bacc as bacc
    import concourse.tile as tile
    import concourse.mybir as mybir
    from concourse.tile import add_dep_helper
    from concourse.masks import make_identity

    f16, f32, i16 = mybir.dt.float16, mybir.dt.float32, mybir.dt.int16
    nc = bacc.Bacc("TRN2", num_devices=N_CORES)

    zT_h = nc.dram_tensor("zt", [LATENT, N], f16, kind="ExternalInput")
    w1_h = nc.dram_tensor("w1", [LATENT, HID], f16, kind="ExternalInput")
    b1_h = nc.dram_tensor("b1", [HID], f32, kind="ExternalInput")
    w2_h = nc.dram_tensor("w2", [HID, 2 * C], f16, kind="ExternalInput")
    b2_h = nc.dram_tensor("b2", [2 * C], f32, kind="ExternalInput")
    wg_h = nc.dram_tensor("wg", [2 * C, 264], f16, kind="ExternalInput")
    bg_h = nc.dram_tensor("bg", [264], f16, kind="ExternalInput")
    w3_h = nc.dram_tensor("w3", [C, LATENT], f16, kind="ExternalInput")
    b3_h = nc.dram_tensor("b3", [LATENT], f32, kind="ExternalInput")
    gidx_h = nc.dram_tensor("gidx", [128, NCH * 8], i16, kind="ExternalInput")
    mask_h = nc.dram_tensor("mask", [128, NCH, W], f16, kind="ExternalInput")
    y_h = nc.dram_tensor("y", [NB, LATENT], f32, kind="ExternalOutput")
    tab = nc.dram_tensor("tab", [N, ROW], f16)

    NSTRIPE = 8
    SB = N // NSTRIPE            # 2500 nodes per stripe
    n_nt = (SB + 127) // 128     # 20 tiles per stripe
    NT = 500                     # matmul column tile
    n_mt = SB // NT              # 5
    NKMAX = int((NWIN * caps).max())

    with tile.TileContext(nc) as tc:
        const = tc.alloc_tile_pool(name="const", bufs=1)

        w1t = const.tile([128, 4, HID], f16)
        nc.sync.dma_start(out=w1t, in_=w1_h.ap().rearrange("(kc k) o -> k kc o", k=128))
        w2t = const.tile([128, 2, 2 * C], f16)
        nc.sync.dma_start(out=w2t, in_=w2_h.ap().rearrange("(kc k) o -> k kc o", k=128))
        wgt = const.tile([128, 4, 264], f16)
        nc.sync.dma_start(out=wgt, in_=wg_h.ap().rearrange("(kc k) o -> k kc o", k=128))
        w3t = const.tile([128, 2, LATENT], f16)
        nc.sync.dma_start(out=w3t, in_=w3_h.ap().rearrange("(kc k) o -> k kc o", k=128))
        b1t = const.tile([128, 2], f32)
        nc.sync.dma_start(out=b1t, in_=b1_h.ap().rearrange("(oc p) -> p oc", p=128))
        b2t = const.tile([128, 4], f32)
        nc.sync.dma_start(out=b2t, in_=b2_h.ap().rearrange("(oc p) -> p oc", p=128))
        b3rep = const.tile([128, LATENT], f32)
        nc.sync.dma_start(out=b3rep, in_=bass.AP(tensor=b3_h, offset=0, ap=[[0, 128], [1, LATENT]]))
        bgrep = const.tile([128, 264], f16)
        nc.sync.dma_start(out=bgrep, in_=bass.AP(tensor=bg_h, offset=0, ap=[[0, 128], [1, 264]]))
        gidx_t = const.tile([128, NCH * 8], i16)
        nc.sync.dma_start(out=gidx_t, in_=gidx_h.ap())
        ident = const.tile([128, 128], f32)
        make_identity(nc, ident)
        identh = const.tile([128, 128], f16)
        nc.vector.tensor_copy(out=identh, in_=ident)

        tab_dmas = []

        # ---------------- MLP phase (all N nodes, 8 stripes) ----------------
        zpool = tc.alloc_tile_pool(name="zpool", bufs=2)
        apool = tc.alloc_tile_pool(name="apool", bufs=2)
        xpool = tc.alloc_tile_pool(name="xpool", bufs=3)
        mps = tc.alloc_tile_pool(name="mps", bufs=3, space="PSUM")
        xps = tc.alloc_tile_pool(name="xps", bufs=3, space="PSUM")

        for s in range(NSTRIPE):
            c0s = s * SB
            zTs = zpool.tile([128, 4, SB], f16, name="zTs")
            nc.sync.dma_start(out=zTs, in_=zT_h.ap()[:, c0s:c0s + SB].rearrange("(kc k) n -> k kc n", k=128))
            x1T = apool.tile([128, 2, SB], f16, name="x1T")
            x2T = apool.tile([128, 4, SB], f16, name="x2T")
            for mt in range(n_mt):
                c0, cn = mt * NT, NT
                for oc in range(2):
                    ps = mps.tile([128, NT], f32, name="mm1", tag="mm")
                    for kc in range(4):
                        nc.tensor.matmul(ps, lhsT=w1t[:, kc, oc * 128:(oc + 1) * 128],
                                         rhs=zTs[:, kc, c0:c0 + cn], start=(kc == 0), stop=(kc == 3))
                    eng = nc.vector if oc == 0 else nc.scalar
                    if eng is nc.vector:
                        nc.vector.tensor_scalar(out=x1T[:, oc, c0:c0 + cn], in0=ps,
                                                scalar1=b1t[:, oc:oc + 1], scalar2=0.0,
                                                op0=mybir.AluOpType.add, op1=mybir.AluOpType.max)
                    else:
                        nc.scalar.activation(out=x1T[:, oc, c0:c0 + cn], in_=ps,
                                             func=mybir.ActivationFunctionType.Relu, bias=b1t[:, oc:oc + 1])
                for oc in range(4):
                    ps = mps.tile([128, NT], f32, name="mm2", tag="mm")
                    for kc in range(2):
                        nc.tensor.matmul(ps, lhsT=w2t[:, kc, oc * 128:(oc + 1) * 128],
                                         rhs=x1T[:, kc, c0:c0 + cn], start=(kc == 0), stop=(kc == 1))
                    if oc % 2 == 0:
                        nc.vector.tensor_scalar(out=x2T[:, oc, c0:c0 + cn], in0=ps,
                                                scalar1=b2t[:, oc:oc + 1], scalar2=0.0,
                                                op0=mybir.AluOpType.add, op1=mybir.AluOpType.max)
                    else:
                        nc.scalar.activation(out=x2T[:, oc, c0:c0 + cn], in_=ps,
                                             func=mybir.ActivationFunctionType.Relu, bias=b2t[:, oc:oc + 1])
            # node-major xpx: lhsT = x2T block [128k, 128 nodes], rhs = wgt
            for t in range(n_nt):
                r0, nr = t * 128, min(128, SB - t * 128)
                xpp = xps.tile([128, 264], f32, name="xpp", tag="xp")
                for kc in range(4):
                    nc.tensor.matmul(xpp[0:nr, :], lhsT=x2T[:, kc, t * 128:t * 128 + nr],
                                     rhs=wgt[:, kc, :], start=(kc == 0), stop=(kc == 3))
                xpn = xpool.tile([128, ROW], f16, name="xpn")
                if nr < 128:
                    nc.vector.memset(xpn, 0.0)
                else:
                    nc.vector.memset(xpn[:, 264:ROW], 0.0)
                nc.vector.tensor_tensor(out=xpn[0:nr, 0:264], in0=xpp[0:nr, :],
                                        in1=bgrep[0:nr, :], op=mybir.AluOpType.add)
                tab_dmas.append(nc.sync.dma_start(
                    out=tab.ap()[c0s + r0:c0s + r0 + nr, :], in_=xpn[0:nr, :]))

        xps.release()
        mps.release()
        xpool.release()
        apool.release()
        zpool.release()

        # ---------------- edge phase ----------------
        gpool = tc.alloc_tile_pool(name="gpool", bufs=4)
        work = tc.alloc_tile_pool(name="work", bufs=2)
        aps_pool = tc.alloc_tile_pool(name="aps", bufs=2, space="PSUM")
        tps = tc.alloc_tile_pool(name="tps", bufs=2, space="PSUM")
        yps_pool = tc.alloc_tile_pool(name="yps", bufs=2, space="PSUM")

        gmark_t = const.tile([128, 2], f32)
        gsems = [nc.alloc_semaphore("gsemA"), nc.alloc_semaphore("gsemB")]
        psems = [nc.alloc_semaphore("psemA"), nc.alloc_semaphore("psemB")]
        wtab = nc.gpsimd.memset(gmark_t[0:1, 0:1], 0)
        for d in tab_dmas:
            add_dep_helper(wtab.ins, d.ins, sync=True, reason="tab writes done")
        prep_state = {"n": [0, 0]}

        def chain(inst):
            return inst

        def prep_sc(sc):
            cap = int(caps[sc])
            k0 = int(sc_base[sc])
            nk = NWIN * cap
            G = gpool.tile([128, NKMAX, ROW], f16, name="G")
            nb = 0
            q = sc % 2
            with tc.tile_critical():
                for c0 in range(0, nk, GMAX):
                    cn = min(GMAX, nk - c0)
                    prep_state["n"][q] += 1
                    nb += 1
                    gi = nc.gpsimd.dma_gather(
                        G[:, c0:c0 + cn, :], tab.ap(),
                        gidx_t[:, (k0 + c0) * 8:(k0 + c0 + cn) * 8],
                        num_idxs=cn * 128, num_idxs_reg=cn * 128,
                        elem_size=ROW, prepare_only=True, sem=gsems[q]).then_inc(psems[q], 1)
                    add_dep_helper(gi.ins, wtab.ins, sync=False, reason="after tab")
                chain(nc.gpsimd.wait_ge(psems[q], prep_state["n"][q]))
                chain(nc.gpsimd.trigger_dma(count=nb))
            return G, prep_state["n"][q]

        def stage_a(sc, G, n_prep):
            cap = int(caps[sc])
            k0 = int(sc_base[sc])
            nk = NWIN * cap
            with tc.tile_critical():
                chain(nc.gpsimd.wait_ge(gsems[sc % 2], 16 * n_prep))
            gmark = chain(nc.gpsimd.memset(gmark_t[0:1, sc % 2:sc % 2 + 1], 0))

            mask_t = work.tile([128, NKMAX, W], f16, name="maskt")
            nc.sync.dma_start(out=mask_t[:, 0:nk, :], in_=mask_h.ap()[:, k0:k0 + nk, :])
            adst_rep = work.tile([128, 128, 2], f16, name="adrep")
            nc.sync.dma_start(out=adst_rep, in_=bass.AP(
                tensor=tab, offset=(sc * 128) * ROW + 260, ap=[[0, 128], [ROW, 128], [1, 2]]))

            # S = mask * exp(lrelu(a_src + a_dst)); both heads in one slab
            lg = work.tile([128, 2, NKMAX, W], f16, name="lg")
            lg2 = work.tile([128, 2, NKMAX, W], f16, name="lg2")
            Sh = work.tile([128, 2, NKMAX, W], f16, name="Sh")
            for h in range(2):
                for wi in range(NWIN):
                    kk0 = wi * cap
                    asrc_b = bass.AP(tensor=G.tensor, offset=G.offset + kk0 * ROW + 258 + h,
                                     ap=[list(G.ap[0]), [ROW, cap], [0, W]])
                    adst_b = bass.AP(tensor=adst_rep.tensor,
                                     offset=adst_rep.offset + (wi * W) * 2 + h,
                                     ap=[list(adst_rep.ap[0]), [0, cap], [2, W]])
                    ad = nc.vector.tensor_tensor(out=lg[:, h, kk0:kk0 + cap, :],
                                                 in0=asrc_b, in1=adst_b,
                                                 op=mybir.AluOpType.add)
                    add_dep_helper(ad.ins, gmark.ins, sync=True, reason="G landed")
            lgw = bass.AP(tensor=lg.tensor, offset=lg.offset,
                          ap=[list(lg.ap[0]), [NKMAX * W, 2], [1, nk * W]])
            lg2w = bass.AP(tensor=lg2.tensor, offset=lg2.offset,
                           ap=[list(lg2.ap[0]), [NKMAX * W, 2], [1, nk * W]])
            shw = bass.AP(tensor=Sh.tensor, offset=Sh.offset,
                          ap=[list(Sh.ap[0]), [NKMAX * W, 2], [1, nk * W]])
            maskw = bass.AP(tensor=mask_t.tensor, offset=mask_t.offset,
                            ap=[list(mask_t.ap[0]), [0, 2], [1, nk * W]])
            nc.vector.tensor_scalar(out=lg2w, in0=lgw, scalar1=NEG, scalar2=None,
                                    op0=mybir.AluOpType.mult)
            nc.vector.tensor_tensor(out=lgw, in0=lgw, in1=lg2w, op=mybir.AluOpType.max)
            nc.scalar.activation(out=lgw, in_=lgw, func=mybir.ActivationFunctionType.Exp)
            nc.vector.tensor_tensor(out=shw, in0=maskw, in1=lgw, op=mybir.AluOpType.mult)

            # aggregation matmuls: S^T @ [xp_h | 1] accumulated per window
            ps = aps_pool.tile([128, 258], f32, name="agg")
            nc.vector.memset(ps, 0.0)
            for k in range(nk):
                wi = k // cap
                w0 = wi * W
                for h in range(2):
                    m1 = nc.tensor.matmul(ps[w0:w0 + W, h * 129:(h + 1) * 129],
                                          lhsT=Sh[:, h, k, :], rhs=G[:, k, h * 129:(h + 1) * 129],
                                          start=False, stop=False,
                                          tile_position=(0, w0), skip_group_check=True)
                    add_dep_helper(m1.ins, gmark.ins, sync=True, reason="G landed pe")

            aslab = work.tile([128, 4], f16, name="aslab")
            nc.sync.dma_start(out=aslab, in_=bass.AP(
                tensor=tab, offset=(sc * 128) * ROW + 258, ap=[[ROW, 128], [1, 4]]))
            xploc = work.tile([128, 2, 128], f16, name="xploc")
            nc.sync.dma_start(out=xploc, in_=bass.AP(
                tensor=tab, offset=(sc * 128) * ROW, ap=[[ROW, 128], [129, 2], [1, 128]]))
            return ps, aslab, xploc

        def stage_b(sc, ps, aslab, xploc):
            nrows = min(128, NB - sc * 128)
            lsf = work.tile([128, 2], f32, name="lsf")
            nc.vector.tensor_tensor(out=lsf, in0=aslab[:, 0:2], in1=aslab[:, 2:4], op=mybir.AluOpType.add)
            lsf2 = work.tile([128, 2], f32, name="lsf2")
            nc.vector.tensor_scalar(out=lsf2, in0=lsf, scalar1=NEG, scalar2=None, op0=mybir.AluOpType.mult)
            nc.vector.tensor_tensor(out=lsf, in0=lsf, in1=lsf2, op=mybir.AluOpType.max)
            wself = work.tile([128, 2], f32, name="wself")
            nc.scalar.activation(out=wself, in_=lsf, func=mybir.ActivationFunctionType.Exp)

            gat = work.tile([128, C], f32, name="gat")
            den = work.tile([128, 2], f32, name="den")
            for h in range(2):
                nc.vector.tensor_scalar(out=gat[:, h * OUT:(h + 1) * OUT],
                                        in0=xploc[:, h, :],
                                        scalar1=wself[:, h:h + 1], scalar2=None,
                                        op0=mybir.AluOpType.mult)
                nc.vector.tensor_tensor(out=gat[:, h * OUT:(h + 1) * OUT],
                                        in0=gat[:, h * OUT:(h + 1) * OUT],
                                        in1=ps[:, h * 129:h * 129 + 128], op=mybir.AluOpType.add)
            nc.vector.tensor_tensor(out=den, in0=wself,
                                    in1=bass.AP(tensor=ps.tensor, offset=ps.offset + 128,
                                                ap=[list(ps.ap[0]), [129, 2]]),
                                    op=mybir.AluOpType.add)
            rden = work.tile([128, 2], f32, name="rden")
            nc.vector.reciprocal(out=rden, in_=den)
            gatn = work.tile([128, C], f16, name="gatn")
            for h in range(2):
                nc.vector.tensor_scalar(out=gatn[:, h * OUT:(h + 1) * OUT],
                                        in0=gat[:, h * OUT:(h + 1) * OUT],
                                        scalar1=rden[:, h:h + 1], scalar2=None,
                                        op0=mybir.AluOpType.mult)

            # ---- transpose + final matmul ----
            gatT = work.tile([128, 2, 128], f16, name="gatT")
            for g in range(2):
                ptt = tps.tile([128, 128], f16, name="gtt", tag="tp")
                nc.tensor.transpose(ptt, gatn[:, g * 128:(g + 1) * 128], identh)
                nc.scalar.copy(gatT[:, g, :], ptt)
            yps = yps_pool.tile([128, LATENT], f32, name="yps")
            for g in range(2):
                nc.tensor.matmul(yps, lhsT=gatT[:, g, :],
                                 rhs=w3t[:, g, :], start=(g == 0), stop=(g == 1))
            ysb = work.tile([128, LATENT], f32, name="ysb")
            nc.vector.tensor_tensor(out=ysb, in0=yps, in1=b3rep, op=mybir.AluOpType.add)
            nc.sync.dma_start(out=y_h.ap()[sc * 128:sc * 128 + nrows, :], in_=ysb[0:nrows, :])

        # software pipeline: prep(sc) | stageA(sc-1) | stageB(sc-2)
        pendA, pendB = None, None
        for sc in range(n_sc):
            G, cum = prep_sc(sc)
            if pendA is not None:
                st = stage_a(*pendA)
                if pendB is not None:
                    stage_b(*pendB)
                pendB = (pendA[0],) + st
            pendA = (sc, G, cum)
        st = stage_a(*pendA)
        stage_b(*pendB)
        stage_b(pendA[0], *st)

        yps_pool.release()
        tps.release()
        aps_pool.release()
        work.release()
        gpool.release()
        const.release()

    nc.compile()
    return nc


def _prepare(inputs):
    z = np.asarray(inputs["z"], np.float32)
    ei = np.asarray(inputs["edge_index"], np.int64)
    W1 = np.asarray(inputs["W1"], np.float32)
    b1 = np.asarray(inputs["b1"], np.float32)
    W2 = np.asarray(inputs["W2"], np.float32)
    b2 = np.asarray(inputs["b2"], np.float32)
    Wg = np.asarray(inputs["Wg"], np.float32)
    att_src = np.asarray(inputs["att_src"], np.float32)
    att_dst = np.asarray(inputs["att_dst"], np.float32)
    bias_g = np.asarray(inputs["bias_g"], np.float32)
    W3 = np.asarray(inputs["W3"], np.float32)
    b3 = np.asarray(inputs["b3"], np.float32)

    N = z.shape[0]
    NB = N // N_CORES
    n_sc = (NB + 127) // 128

    # column layout: [Wg_h0 | ones | Wg_h1 | ones | vs | vd | pad2]
    KG = Wg.shape[0]
    vs = np.zeros((KG, 2), np.float32)
    vd = np.zeros((KG, 2), np.float32)
    for h in range(HEADS):
        vs[:, h] = Wg[:, h * OUT:(h + 1) * OUT] @ att_src[h]
        vd[:, h] = Wg[:, h * OUT:(h + 1) * OUT] @ att_dst[h]
    zcol = np.zeros((KG, 1), np.float32)
    wg_ext = np.concatenate([Wg[:, 0:128], zcol, Wg[:, 128:256], zcol,
                             vs, vd, np.zeros((KG, 2), np.float32)], axis=1)
    ab_src = np.array([bias_g[h * OUT:(h + 1) * OUT] @ att_src[h] for h in range(HEADS)], np.float32)
    ab_dst = np.array([bias_g[h * OUT:(h + 1) * OUT] @ att_dst[h] for h in range(HEADS)], np.float32)
    bg_ext = np.concatenate([bias_g[0:128], [1.0], bias_g[128:256], [1.0],
                             ab_src, ab_dst, np.zeros(2, np.float32)]).astype(np.float32)

    gidx_all, mask_all, caps, sc_base, NCH = _plan(ei[0], ei[1], N, NB, n_sc)

    nc = _build(N, NB, n_sc, caps, sc_base, NCH)

    z16 = z.astype(np.float16)
    in_maps = []
    for c in range(N_CORES):
        zrot = np.concatenate([z16[c * NB:], z16[:c * NB]], axis=0)
        in_maps.append({
            "zt": np.ascontiguousarray(zrot.T),
            "w1": W1.astype(np.float16), "b1": b1,
            "w2": W2.astype(np.float16), "b2": b2,
            "wg": wg_ext.astype(np.float16), "bg": bg_ext.astype(np.float16),
            "w3": W3.astype(np.float16), "b3": b3,
            "gidx": gidx_all[c], "mask": mask_all[c],
        })
    return nc, in_maps


def kernel(**inputs):
    import sys
    if '/opt/trn_rl_repo' not in sys.path:
        sys.path.insert(0, '/opt/trn_rl_repo')
    from concourse.bass_utils import run_bass_kernel_spmd

    nc, in_maps = _prepare(inputs)
    res = run_bass_kernel_spmd(nc, in_maps, list(range(N_CORES)))
    global LAST_RESULT
    LAST_RESULT = res
    y = np.concatenate([res.results[c]["y"] for c in range(N_CORES)], axis=0)
    return y.astype(np.float32)
